# revision 13
# baseline (speedup 1.0000x reference)
"""AttentionWeightedAverage distributed Trainium2 kernel.

Reference computation (all f32):
    s     = wv @ v + wg @ h          # (512, 384) + (512, 1) broadcast
    t     = tanh(s)                  # (512, 384)
    z     = wh @ t                   # (384, 384)
    alpha = softmax(z, axis=-1)      # (384, 384)
    out[i, j, l] = v[j, l] * alpha[i, j]   # (384, 384, 384)

The output (226 MB f32) dwarfs the inputs (~2.5 MB), so the kernel is
bound by per-core HBM write bandwidth. Sharding: every core gets the
full (small) weights and computes s/t redundantly; core m owns rows
i in [m*48, (m+1)*48) of z/alpha and writes that contiguous slice of
the output. No collectives.

Design (v4):
- The output is stored as bf16 and upcast to f32 on the host. The
  correctness gate is scale-relative 2e-2; bf16 rounding of v and of
  the product adds ~4e-3. Store stream: 14.2 MB/core -> ~38 us at the
  per-core HBM write limit. That stream is the roofline; everything
  else exists to start it early and keep it fed.
- The broadcast source v3 uses layout B (v3[p, c*384+l] = v[3p+c, l])
  so each partition's 3 output rows are consecutive -> 2304 B
  contiguous HBM runs per store descriptor row. 768 B runs (storing
  straight from the matmul layout) measurably cost ~15% of store
  bandwidth (packetization + per-descriptor metadata overhead).
- Input loads are HBM-read-bound (~260 GB/s effective with all 8
  cores loading at once), so wg - the biggest input, and the head of
  the longest dependency chain (wg -> gh -> s -> tanh -> z) - ships
  as fp8 e3m4 (4 mantissa bits; wg ~ +-0.3 so range is fine) and is
  upcast to bf16 on the DVE after landing. Only gh is perturbed
  (~1% elementwise), and the j-constant part of the resulting z error
  cancels in softmax; measured end-to-end error stays ~6e-3.
- Loads are chunked across the two HWDGE rings roughly evenly
  (descriptor-gen is ~0.6 us per dma_start, serialized per ring), in
  dependency order; the PE executes in issue order, so matmul
  emission follows the arrival schedule: s-k0 pass, ghT k01, s-k1
  pass, ghT k23, s-k2 pass, then the rank-1 gh pass (lhsT = ghT
  slice, rhs = ones row) closing each accumulation chained with its
  tanh.
- z/softmax/alpha-transpose run in two 24-row halves; half 1 is
  emitted right after the first store block so ACT's exp is not
  queued behind store work.
- All store dma_starts go on the sync ring (keeping the ACT
  sequencer free for its multiply share); ACT takes rows i%4==3 of
  the first 36 rows only, so the final blocks are never gated on
  ACT's slower (~613 ns) ops; everything else runs on DVE (~229 ns
  per 128x384 bf16 op - the [P,1] f32 scalar occupies a read port,
  so the 4x DVE mode is unavailable and 2x is the cap).
- Throwaway matmuls on zeroed tiles keep the PE busy from kernel
  start until the first input chunk lands: the HAM clock gate needs
  ~3.4 us of sustained activity to double the PE clock and
  re-throttles after ~3.4 us idle.
- softmax skips the max-subtraction: |z| stays far from f32 exp
  overflow and softmax is shift-invariant. The exp's accum_out gives
  the row sums for free.

Per-core SBUF layouts (P = 128 partitions):
    wvb  (128, 3*896) bf16: per k: [wvT_k | vb_k];
         wvT_k[p, e] = wv[e, k*128+p], vb_k[p, l] = v[k*128+p, l]
    wg8  (128, 2048) fp8e3m4 -> wg_sb bf16: wg_sb[p, k*512+e] =
         wg[e, k*128+p]
    hwhT (128, 4+192) bf16: [h3 | whT3]; h3[p,k] = h[k*128+p],
         whT3[p, k*48+i] = wh[m*48+i, k*128+p]
    v3   (128, 1152) bf16: v3[p, c*384+l] = v[3p+c, l]
"""

import numpy as np

import concourse.bacc as bacc
import concourse.mybir as mybir
from concourse import masks
from concourse.bass_utils import run_bass_kernel_spmd
from concourse.tile import TileContext

F32 = mybir.dt.float32
BF16 = mybir.dt.bfloat16
AF = mybir.ActivationFunctionType

NCORES = 8
L = 384          # vfeat_len == vfeat_dim
E = 512          # embed dim
IPC = L // NCORES  # 48 output rows per core
P = 128
CJ = L // P      # 3 chunks over the j axis
KV = L // P      # 3 contraction chunks for wv@v
KE = E // P      # 4 contraction chunks over embed dim
WVB = E + L      # fused [wvT_k | vb_k] chunk width
IPB = 2          # output rows batched per store DMA
OUT_BUFS = 10    # in-flight output tiles
HZ = IPC // 2    # z/softmax half size
NWARM = 7        # PE warmup matmuls (until the first input chunks land)
ACT_ROWS = 40    # ACT multiply share upper bound (rows 6..38, i%4==2)


def _build_nc() -> bacc.Bacc:
    nc = bacc.Bacc()

    wvb_d = nc.declare_dram_parameter("wvb", [P, KV * WVB], BF16, isOutput=False)
    hwg_d = nc.declare_dram_parameter("hwg", [P, KE + KE * E], BF16, isOutput=False)
    whT3_d = nc.declare_dram_parameter("whT3", [P, KE * IPC], BF16, isOutput=False)
    v3_d = nc.declare_dram_parameter("v3", [P, CJ * L], BF16, isOutput=False)
    out_d = nc.declare_dram_parameter("out", [IPC, L, L], BF16, isOutput=True)

    with TileContext(nc) as tc:
        with (
            tc.tile_pool(name="const", bufs=1) as cpool,
            tc.tile_pool(name="work", bufs=2) as wpool,
            tc.tile_pool(name="psum", bufs=2, space="PSUM") as ppool,
            tc.tile_pool(name="outp", bufs=OUT_BUFS) as opool,
        ):
            # ---- input loads, chunked per HWDGE ring in dependency
            # order. scalar ring: h+whT, wg8 halves, wvb k2.
            # sync ring: wvb k0, k1, v3 (then all store descs).
            hwg_sb = cpool.tile([P, KE + KE * E], BF16)
            nc.scalar.dma_start(out=hwg_sb[:], in_=hwg_d[:])
            whT_sb = cpool.tile([P, KE * IPC], BF16)
            nc.scalar.dma_start(out=whT_sb[:], in_=whT3_d[:])
            v_sb = cpool.tile([P, CJ * L], BF16)
            nc.scalar.dma_start(out=v_sb[:], in_=v3_d[:])
            wvb_sb = cpool.tile([P, KV * WVB], BF16)
            for k in range(KV):
                nc.sync.dma_start(
                    out=wvb_sb[:, k * WVB : (k + 1) * WVB],
                    in_=wvb_d[:, k * WVB : (k + 1) * WVB],
                )

            h_sb = hwg_sb[:, 0:KE]
            wg_sb = hwg_sb[:, KE:]

            ident = cpool.tile([IPC, IPC], F32)
            masks.make_identity(nc, ident[:])
            ones_row = cpool.tile([1, L], BF16)
            nc.gpsimd.memset(ones_row[:], 1.0)

            # Keep the PE busy from kernel start until the input chunks
            # land (HAM clock warmup; see module docstring).
            warm_w = cpool.tile([P, P], BF16)
            warm_x = cpool.tile([P, L], BF16)
            nc.gpsimd.memset(warm_w[:], 0.0)
            nc.gpsimd.memset(warm_x[:], 0.0)
            warm_ps = ppool.tile([P, L], F32, tag="s_ps", bufs=KE)
            for w in range(NWARM):
                nc.tensor.matmul(
                    warm_ps[:],
                    lhsT=warm_w[:],
                    rhs=warm_x[:],
                    start=(w == 0),
                    stop=(w == NWARM - 1),
                )

            # ---- t = tanh(wv @ v + gh . 1^T), gh = wg @ h
            # t3[p, mc*384+j] = t[mc*128+p, j]
            t3 = cpool.tile([P, KE * L], BF16)
            s_ps = [
                ppool.tile([P, L], F32, tag="s_ps", bufs=KE, name=f"s_ps{mc}")
                for mc in range(KE)
            ]
            ghT_ps = ppool.tile([1, E], F32, tag="zg", bufs=2)

            def ghT_chunk(k):
                nc.tensor.matmul(
                    ghT_ps[:],
                    lhsT=h_sb[:, k : k + 1],
                    rhs=wg_sb[:, k * E : (k + 1) * E],
                    start=(k == 0),
                    stop=(k == KE - 1),
                )

            def s_pass(k, start):
                for mc in range(KE):
                    nc.tensor.matmul(
                        s_ps[mc][:],
                        lhsT=wvb_sb[:, k * WVB + mc * P : k * WVB + (mc + 1) * P],
                        rhs=wvb_sb[:, k * WVB + E : (k + 1) * WVB],
                        start=start,
                        stop=False,
                    )

            for k in range(KE):
                ghT_chunk(k)
            s_pass(0, start=True)
            s_pass(1, start=False)
            s_pass(2, start=False)
            ghT_sb = wpool.tile([1, E], BF16)
            nc.vector.tensor_copy(ghT_sb[:], ghT_ps[:])
            for mc in range(KE):
                nc.tensor.matmul(
                    s_ps[mc][:],
                    lhsT=ghT_sb[:, mc * P : (mc + 1) * P],
                    rhs=ones_row[:],
                    start=False,
                    stop=True,
                )
                nc.scalar.activation(
                    t3[:, mc * L : (mc + 1) * L], s_ps[mc][:], AF.Tanh
                )

            # ---- z rows, softmax, and transpose in two 24-row halves;
            # the first store blocks are emitted right after half 0 so
            # the store stream starts early.
            alphaT = ppool.tile([P, CJ * IPC], F32, tag="aT", bufs=1)
            alphaT_sb = wpool.tile([P, CJ * IPC], F32)

            from concourse.tile_rust import add_dep_helper

            def z_half(hh, after=None):
                r0 = hh * HZ
                z_h = ppool.tile([HZ, L], F32, tag="zg", bufs=2)
                for k in range(KE):
                    mm = nc.tensor.matmul(
                        z_h[:],
                        lhsT=whT_sb[:, k * IPC + r0 : k * IPC + r0 + HZ],
                        rhs=t3[:, k * L : (k + 1) * L],
                        start=(k == 0),
                        stop=(k == KE - 1),
                    )
                    if k == 0 and after is not None:
                        # keep this half's PE work behind the previous
                        # half's alpha transposes
                        add_dep_helper(
                            mm.ins, after.ins, reason="z halves in order"
                        )
                # softmax (no max shift; fused row sums)
                e_h = wpool.tile([HZ, L], F32, tag="e_h")
                rsum_h = wpool.tile([HZ, 1], F32, tag="rsum_h")
                exp_i = nc.scalar.activation(
                    e_h[:], z_h[:], AF.Exp, accum_out=rsum_h[:]
                )
                rinv_h = wpool.tile([HZ, 1], F32, tag="rinv_h")
                nc.vector.reciprocal(rinv_h[:], rsum_h[:])
                # alphaT[p, c*48+i] = alpha[i, 3p+c]; the DVE normalize
                # also performs the stride-3 column gather (j = 3p+c) so
                # the PE transpose reads a contiguous slice.
                alpha_h = wpool.tile([HZ, L], F32, tag="alpha_h")
                last_t = None
                for c in range(CJ):
                    nc.vector.tensor_scalar_mul(
                        alpha_h[:, c * P : (c + 1) * P],
                        e_h.rearrange("i (p c) -> c i p", c=CJ)[c],
                        rinv_h[:],
                    )
                    last_t = nc.tensor.transpose(
                        alphaT[:, c * IPC + r0 : c * IPC + r0 + HZ],
                        alpha_h[:, c * P : (c + 1) * P],
                        ident[0:HZ, 0:HZ],
                    )
                    # SBUF mirror, used by the ACT-routed multiplies
                    # (ACT requires an SBUF scale); off the critical path
                    nc.vector.tensor_copy(
                        alphaT_sb[:, c * IPC + r0 : c * IPC + r0 + HZ],
                        alphaT[:, c * IPC + r0 : c * IPC + r0 + HZ],
                    )
                return last_t

            def emit_block(ib, nb, ring=None):
                ot = opool.tile([P, IPB * CJ * L], BF16, tag="ot")
                for t in range(nb):
                    i = ib + t
                    for c in range(CJ):
                        dst = ot[:, (t * CJ + c) * L : (t * CJ + c + 1) * L]
                        src = v_sb[:, c * L : (c + 1) * L]
                        if (i == 0 and c == 1) or (
                            6 <= i < ACT_ROWS and i % 4 == 2
                        ):
                            nc.scalar.mul(
                                dst, src,
                                alphaT_sb[:, c * IPC + i : c * IPC + i + 1],
                            )
                        else:
                            nc.vector.tensor_scalar_mul(
                                dst, src,
                                alphaT[:, c * IPC + i : c * IPC + i + 1],
                            )
                # out row j = 3p+c -> 2304 B contiguous runs per (p, t)
                dram_ap = out_d[ib : ib + nb].rearrange(
                    "t (p c) l -> p t c l", p=P, c=CJ
                )
                sb_ap = ot[:, 0 : nb * CJ * L].rearrange(
                    "p (t c l) -> p t c l", t=nb, c=CJ
                )
                (ring or nc.sync).dma_start(out=dram_ap, in_=sb_ap)

            blocks = [(0, 1)] + [
                (ib, IPB) for ib in range(1, IPC - 1, IPB)
            ] + [(IPC - 1, 1)]

            tr0 = z_half(0)

            # first row: one multiply + one store dma per c-chunk (on
            # the otherwise-idle scalar ring) so bytes hit HBM asap
            ot0 = opool.tile([P, IPB * CJ * L], BF16, tag="ot")
            for c in range(CJ):
                dst = ot0[:, c * L : (c + 1) * L]
                src_ = v_sb[:, c * L : (c + 1) * L]
                if c == 1:
                    nc.scalar.mul(
                        dst, src_, alphaT_sb[:, c * IPC : c * IPC + 1]
                    )
                else:
                    nc.vector.tensor_scalar_mul(
                        dst, src_, alphaT[:, c * IPC : c * IPC + 1]
                    )
                nc.scalar.dma_start(
                    out=out_d[0:1].rearrange("t (p c) l -> p t c l", p=P, c=CJ)[
                        :, :, c : c + 1
                    ],
                    in_=ot0[:, c * L : (c + 1) * L].rearrange(
                        "p (t c l) -> p t c l", t=1, c=1
                    ),
                )

            first = True
            for ib, nb in blocks[1:]:
                if first:
                    # right after the first block so ACT's exp is not
                    # queued behind store work and ACT multiplies
                    z_half(1, after=tr0)
                    first = False
                # the last few blocks alternate rings so their packets
                # interleave across the SDMA queues (tail-skew hedge)
                ring = nc.scalar if ib >= 41 and (ib // 2) % 2 == 0 else None
                emit_block(ib, nb, ring)

    nc.compile()
    return nc


def _prep_inputs(h, v, wh, wv, wg):
    """Host-side relayout into the per-core SBUF-friendly layouts."""
    import ml_dtypes

    h = np.ascontiguousarray(h, dtype=np.float32)
    v = np.ascontiguousarray(v, dtype=np.float32)
    wh = np.ascontiguousarray(wh, dtype=np.float32)
    wv = np.ascontiguousarray(wv, dtype=np.float32)
    wg = np.ascontiguousarray(wg, dtype=np.float32)

    def bf16(x):
        return np.ascontiguousarray(x.astype(ml_dtypes.bfloat16))

    # v3 (broadcast source): layout B, v3[p, c*384+l] = v[3p+c, l]
    v3 = bf16(v.reshape(P, CJ * L))
    # fused [wvT_k | vb_k] chunks: wvT_k[p, e] = wv[e, k*128+p],
    # vb_k[p, l] = v[k*128+p, l]
    wvT3 = wv.T.reshape(KV, P, E)
    vA = v.reshape(KV, P, L)
    wvb = bf16(
        np.concatenate(
            [np.concatenate([wvT3[k], vA[k]], axis=1) for k in range(KV)],
            axis=1,
        )
    )
    wgT3 = wg.T.reshape(KE, P, E).transpose(1, 0, 2).reshape(P, KE * E)
    hwg = bf16(np.concatenate([h.reshape(KE, P).T, wgT3], axis=1))

    in_maps = []
    for m in range(NCORES):
        whm = wh[m * IPC : (m + 1) * IPC]  # (48, 512)
        whT3 = bf16(
            whm.T.reshape(KE, P, IPC).transpose(1, 0, 2).reshape(P, KE * IPC)
        )
        in_maps.append(
            {
                "wvb": wvb,
                "hwg": hwg,
                "whT3": whT3,
                "v3": v3,
            }
        )
    return in_maps


_NC_CACHE = []


def _run(inputs: dict, trace: bool = False, **kw):
    if not _NC_CACHE:
        _NC_CACHE.append(_build_nc())
    nc = _NC_CACHE[0]
    in_maps = _prep_inputs(**inputs)
    res = run_bass_kernel_spmd(
        nc, in_maps, core_ids=list(range(NCORES)), trace=trace, **kw
    )
    out = np.concatenate(
        [r["out"].astype(np.float32) for r in res.results], axis=0
    )
    return out, res


def kernel(h, v, wh, wv, wg):
    out, _ = _run({"h": h, "v": v, "wh": wh, "wv": wv, "wg": wg})
    return out


# revision 14
# speedup vs baseline: 1.0394x; 1.0394x over previous
"""AttentionWeightedAverage distributed Trainium2 kernel.

Reference computation (all f32):
    s     = wv @ v + wg @ h          # (512, 384) + (512, 1) broadcast
    t     = tanh(s)                  # (512, 384)
    z     = wh @ t                   # (384, 384)
    alpha = softmax(z, axis=-1)      # (384, 384)
    out[i, j, l] = v[j, l] * alpha[i, j]   # (384, 384, 384)

The output (226 MB f32) dwarfs the inputs (~2.5 MB), so the kernel is
bound by per-core HBM write bandwidth. Sharding: every core gets the
full (small) weights and computes s/t redundantly; core m owns rows
i in [m*48, (m+1)*48) of z/alpha and writes that contiguous slice of
the output. No collectives.

Design (v4):
- The output is stored as bf16 and upcast to f32 on the host. The
  correctness gate is scale-relative 2e-2; bf16 rounding of v and of
  the product adds ~4e-3. Store stream: 14.2 MB/core -> ~38 us at the
  per-core HBM write limit. That stream is the roofline; everything
  else exists to start it early and keep it fed.
- The broadcast source v3 uses layout B (v3[p, c*384+l] = v[3p+c, l])
  so each partition's 3 output rows are consecutive -> 2304 B
  contiguous HBM runs per store descriptor row. 768 B runs (storing
  straight from the matmul layout) measurably cost ~15% of store
  bandwidth (packetization + per-descriptor metadata overhead).
- Input loads are HBM-read-bound (~260 GB/s effective with all 8
  cores loading at once), so wg - the biggest input, and the head of
  the longest dependency chain (wg -> gh -> s -> tanh -> z) - ships
  as fp8 e3m4 (4 mantissa bits; wg ~ +-0.3 so range is fine) and is
  upcast to bf16 on the DVE after landing. Only gh is perturbed
  (~1% elementwise), and the j-constant part of the resulting z error
  cancels in softmax; measured end-to-end error stays ~6e-3.
- Loads are chunked across the two HWDGE rings roughly evenly
  (descriptor-gen is ~0.6 us per dma_start, serialized per ring), in
  dependency order; the PE executes in issue order, so matmul
  emission follows the arrival schedule: s-k0 pass, ghT k01, s-k1
  pass, ghT k23, s-k2 pass, then the rank-1 gh pass (lhsT = ghT
  slice, rhs = ones row) closing each accumulation chained with its
  tanh.
- z/softmax/alpha-transpose run in two 24-row halves; half 1 is
  emitted right after the first store block so ACT's exp is not
  queued behind store work.
- All store dma_starts go on the sync ring (keeping the ACT
  sequencer free for its multiply share); ACT takes rows i%4==3 of
  the first 36 rows only, so the final blocks are never gated on
  ACT's slower (~613 ns) ops; everything else runs on DVE (~229 ns
  per 128x384 bf16 op - the [P,1] f32 scalar occupies a read port,
  so the 4x DVE mode is unavailable and 2x is the cap).
- Throwaway matmuls on zeroed tiles keep the PE busy from kernel
  start until the first input chunk lands: the HAM clock gate needs
  ~3.4 us of sustained activity to double the PE clock and
  re-throttles after ~3.4 us idle.
- softmax skips the max-subtraction: |z| stays far from f32 exp
  overflow and softmax is shift-invariant. The exp's accum_out gives
  the row sums for free.

Per-core SBUF layouts (P = 128 partitions):
    wvb  (128, 3*896) bf16: per k: [wvT_k | vb_k];
         wvT_k[p, e] = wv[e, k*128+p], vb_k[p, l] = v[k*128+p, l]
    wg8  (128, 2048) fp8e3m4 -> wg_sb bf16: wg_sb[p, k*512+e] =
         wg[e, k*128+p]
    hwhT (128, 4+192) bf16: [h3 | whT3]; h3[p,k] = h[k*128+p],
         whT3[p, k*48+i] = wh[m*48+i, k*128+p]
    v3   (128, 1152) bf16: v3[p, c*384+l] = v[3p+c, l]
"""

import numpy as np

import concourse.bacc as bacc
import concourse.mybir as mybir
from concourse import masks
from concourse.bass_utils import run_bass_kernel_spmd
from concourse.tile import TileContext

F32 = mybir.dt.float32
BF16 = mybir.dt.bfloat16
AF = mybir.ActivationFunctionType

NCORES = 8
L = 384          # vfeat_len == vfeat_dim
E = 512          # embed dim
IPC = L // NCORES  # 48 output rows per core
P = 128
CJ = L // P      # 3 chunks over the j axis
KV = L // P      # 3 contraction chunks for wv@v
KE = E // P      # 4 contraction chunks over embed dim
WVB = E + L      # fused [wvT_k | vb_k] chunk width
IPB = 2          # output rows batched per store DMA
OUT_BUFS = 10    # in-flight output tiles
HZ = IPC // 2    # z/softmax half size
NWARM = 7        # PE warmup matmuls (until the first input chunks land)
ACT_ROWS = 40    # ACT multiply share upper bound (rows 6..38, i%4==2)


def _build_nc() -> bacc.Bacc:
    nc = bacc.Bacc()

    wvb_d = nc.declare_dram_parameter("wvb", [P, KV * WVB], BF16, isOutput=False)
    hwg_d = nc.declare_dram_parameter("hwg", [P, KE + KE * E], BF16, isOutput=False)
    whT3_d = nc.declare_dram_parameter("whT3", [P, KE * IPC], BF16, isOutput=False)
    v3_d = nc.declare_dram_parameter("v3", [P, CJ * L], BF16, isOutput=False)
    out_d = nc.declare_dram_parameter("out", [IPC, L, L], BF16, isOutput=True)

    with TileContext(nc) as tc:
        with (
            tc.tile_pool(name="const", bufs=1) as cpool,
            tc.tile_pool(name="work", bufs=2) as wpool,
            tc.tile_pool(name="psum", bufs=2, space="PSUM") as ppool,
            tc.tile_pool(name="outp", bufs=OUT_BUFS) as opool,
        ):
            # ---- input loads, chunked per HWDGE ring in dependency
            # order. scalar ring: h+whT, wg8 halves, wvb k2.
            # sync ring: wvb k0, k1, v3 (then all store descs).
            hwg_sb = cpool.tile([P, KE + KE * E], BF16)
            nc.scalar.dma_start(out=hwg_sb[:], in_=hwg_d[:])
            whT_sb = cpool.tile([P, KE * IPC], BF16)
            nc.scalar.dma_start(out=whT_sb[:], in_=whT3_d[:])
            v_sb = cpool.tile([P, CJ * L], BF16)
            nc.scalar.dma_start(out=v_sb[:], in_=v3_d[:])
            wvb_sb = cpool.tile([P, KV * WVB], BF16)
            for k in range(KV):
                nc.sync.dma_start(
                    out=wvb_sb[:, k * WVB : (k + 1) * WVB],
                    in_=wvb_d[:, k * WVB : (k + 1) * WVB],
                )

            h_sb = hwg_sb[:, 0:KE]
            wg_sb = hwg_sb[:, KE:]

            ident = cpool.tile([IPC, IPC], F32)
            masks.make_identity(nc, ident[:])
            ones_row = cpool.tile([1, L], BF16)
            nc.gpsimd.memset(ones_row[:], 1.0)

            # Keep the PE busy from kernel start until the input chunks
            # land (HAM clock warmup; see module docstring).
            warm_w = cpool.tile([P, P], BF16)
            warm_x = cpool.tile([P, L], BF16)
            nc.gpsimd.memset(warm_w[:], 0.0)
            nc.gpsimd.memset(warm_x[:], 0.0)
            warm_ps = ppool.tile([P, L], F32, tag="s_ps", bufs=KE)
            for w in range(NWARM):
                nc.tensor.matmul(
                    warm_ps[:],
                    lhsT=warm_w[:],
                    rhs=warm_x[:],
                    start=(w == 0),
                    stop=(w == NWARM - 1),
                )

            # ---- t = tanh(wv @ v + gh . 1^T), gh = wg @ h
            # t3[p, mc*384+j] = t[mc*128+p, j]
            t3 = cpool.tile([P, KE * L], BF16)
            s_ps = [
                ppool.tile([P, L], F32, tag="s_ps", bufs=KE, name=f"s_ps{mc}")
                for mc in range(KE)
            ]
            ghT_ps = ppool.tile([1, E], F32, tag="zg", bufs=2)

            def ghT_chunk(k):
                nc.tensor.matmul(
                    ghT_ps[:],
                    lhsT=h_sb[:, k : k + 1],
                    rhs=wg_sb[:, k * E : (k + 1) * E],
                    start=(k == 0),
                    stop=(k == KE - 1),
                )

            def s_pass(k, start):
                for mc in range(KE):
                    nc.tensor.matmul(
                        s_ps[mc][:],
                        lhsT=wvb_sb[:, k * WVB + mc * P : k * WVB + (mc + 1) * P],
                        rhs=wvb_sb[:, k * WVB + E : (k + 1) * WVB],
                        start=start,
                        stop=False,
                    )

            for k in range(KE):
                ghT_chunk(k)
            s_pass(0, start=True)
            s_pass(1, start=False)
            s_pass(2, start=False)
            ghT_sb = wpool.tile([1, E], BF16)
            nc.vector.tensor_copy(ghT_sb[:], ghT_ps[:])
            for mc in range(KE):
                nc.tensor.matmul(
                    s_ps[mc][:],
                    lhsT=ghT_sb[:, mc * P : (mc + 1) * P],
                    rhs=ones_row[:],
                    start=False,
                    stop=True,
                )
                nc.scalar.activation(
                    t3[:, mc * L : (mc + 1) * L], s_ps[mc][:], AF.Tanh
                )

            # ---- z rows, softmax, and transpose in two 24-row halves;
            # the first store blocks are emitted right after half 0 so
            # the store stream starts early.
            alphaT = wpool.tile([P, CJ * IPC], F32)

            from concourse.tile_rust import add_dep_helper

            def z_half(hh, after=None):
                r0 = hh * HZ
                z_h = ppool.tile([HZ, L], F32, tag="zg", bufs=2)
                for k in range(KE):
                    mm = nc.tensor.matmul(
                        z_h[:],
                        lhsT=whT_sb[:, k * IPC + r0 : k * IPC + r0 + HZ],
                        rhs=t3[:, k * L : (k + 1) * L],
                        start=(k == 0),
                        stop=(k == KE - 1),
                    )
                    if k == 0 and after is not None:
                        # keep this half's PE work behind the previous
                        # half's alpha transposes
                        add_dep_helper(
                            mm.ins, after.ins, reason="z halves in order"
                        )
                # softmax (no max shift; fused row sums)
                e_h = wpool.tile([HZ, L], F32, tag="e_h")
                rsum_h = wpool.tile([HZ, 1], F32, tag="rsum_h")
                exp_i = nc.scalar.activation(
                    e_h[:], z_h[:], AF.Exp, accum_out=rsum_h[:]
                )
                rinv_h = wpool.tile([HZ, 1], F32, tag="rinv_h")
                nc.vector.reciprocal(rinv_h[:], rsum_h[:])
                # alphaT[p, c*48+i] = alpha[i, 3p+c]; the DVE normalize
                # also performs the stride-3 column gather (j = 3p+c) so
                # the PE transpose reads a contiguous slice.
                alpha_h = wpool.tile([HZ, L], F32, tag="alpha_h")
                last_t = None
                for c in range(CJ):
                    nc.vector.tensor_scalar_mul(
                        alpha_h[:, c * P : (c + 1) * P],
                        e_h.rearrange("i (p c) -> c i p", c=CJ)[c],
                        rinv_h[:],
                    )
                    at_ps = ppool.tile([P, HZ], F32, tag="at_ps")
                    last_t = nc.tensor.transpose(
                        at_ps[:],
                        alpha_h[:, c * P : (c + 1) * P],
                        ident[0:HZ, 0:HZ],
                    )
                    nc.vector.tensor_copy(
                        alphaT[:, c * IPC + r0 : c * IPC + r0 + HZ], at_ps[:]
                    )
                return last_t

            def emit_block(ib, nb, ring=None):
                ot = opool.tile([P, IPB * CJ * L], BF16, tag="ot")
                for t in range(nb):
                    i = ib + t
                    for c in range(CJ):
                        dst = ot[:, (t * CJ + c) * L : (t * CJ + c + 1) * L]
                        src = v_sb[:, c * L : (c + 1) * L]
                        sc = alphaT[:, c * IPC + i : c * IPC + i + 1]
                        if (i == 0 and c == 1) or (
                            6 <= i < ACT_ROWS and i % 4 == 2
                        ):
                            nc.scalar.mul(dst, src, sc)
                        else:
                            nc.vector.tensor_scalar_mul(dst, src, sc)
                # out row j = 3p+c -> 2304 B contiguous runs per (p, t)
                dram_ap = out_d[ib : ib + nb].rearrange(
                    "t (p c) l -> p t c l", p=P, c=CJ
                )
                sb_ap = ot[:, 0 : nb * CJ * L].rearrange(
                    "p (t c l) -> p t c l", t=nb, c=CJ
                )
                (ring or nc.sync).dma_start(out=dram_ap, in_=sb_ap)

            blocks = [(0, 1)] + [
                (ib, IPB) for ib in range(1, IPC - 1, IPB)
            ] + [(IPC - 1, 1)]

            tr0 = z_half(0)

            # first row: one multiply + one store dma per c-chunk (on
            # the otherwise-idle scalar ring) so bytes hit HBM asap
            ot0 = opool.tile([P, IPB * CJ * L], BF16, tag="ot")
            for c in range(CJ):
                dst = ot0[:, c * L : (c + 1) * L]
                src_ = v_sb[:, c * L : (c + 1) * L]
                sc = alphaT[:, c * IPC : c * IPC + 1]
                if c == 1:
                    nc.scalar.mul(dst, src_, sc)
                else:
                    nc.vector.tensor_scalar_mul(dst, src_, sc)
                nc.scalar.dma_start(
                    out=out_d[0:1].rearrange("t (p c) l -> p t c l", p=P, c=CJ)[
                        :, :, c : c + 1
                    ],
                    in_=ot0[:, c * L : (c + 1) * L].rearrange(
                        "p (t c l) -> p t c l", t=1, c=1
                    ),
                )

            first = True
            for ib, nb in blocks[1:]:
                if first:
                    # right after the first block so ACT's exp is not
                    # queued behind store work and ACT multiplies
                    z_half(1, after=tr0)
                    first = False
                # the last few blocks alternate rings so their packets
                # interleave across the SDMA queues (tail-skew hedge)
                ring = nc.scalar if ib >= 41 and (ib // 2) % 2 == 0 else None
                emit_block(ib, nb, ring)

    nc.compile()
    return nc


def _prep_inputs(h, v, wh, wv, wg):
    """Host-side relayout into the per-core SBUF-friendly layouts."""
    import ml_dtypes

    h = np.ascontiguousarray(h, dtype=np.float32)
    v = np.ascontiguousarray(v, dtype=np.float32)
    wh = np.ascontiguousarray(wh, dtype=np.float32)
    wv = np.ascontiguousarray(wv, dtype=np.float32)
    wg = np.ascontiguousarray(wg, dtype=np.float32)

    def bf16(x):
        return np.ascontiguousarray(x.astype(ml_dtypes.bfloat16))

    # v3 (broadcast source): layout B, v3[p, c*384+l] = v[3p+c, l]
    v3 = bf16(v.reshape(P, CJ * L))
    # fused [wvT_k | vb_k] chunks: wvT_k[p, e] = wv[e, k*128+p],
    # vb_k[p, l] = v[k*128+p, l]
    wvT3 = wv.T.reshape(KV, P, E)
    vA = v.reshape(KV, P, L)
    wvb = bf16(
        np.concatenate(
            [np.concatenate([wvT3[k], vA[k]], axis=1) for k in range(KV)],
            axis=1,
        )
    )
    wgT3 = wg.T.reshape(KE, P, E).transpose(1, 0, 2).reshape(P, KE * E)
    hwg = bf16(np.concatenate([h.reshape(KE, P).T, wgT3], axis=1))

    in_maps = []
    for m in range(NCORES):
        whm = wh[m * IPC : (m + 1) * IPC]  # (48, 512)
        whT3 = bf16(
            whm.T.reshape(KE, P, IPC).transpose(1, 0, 2).reshape(P, KE * IPC)
        )
        in_maps.append(
            {
                "wvb": wvb,
                "hwg": hwg,
                "whT3": whT3,
                "v3": v3,
            }
        )
    return in_maps


_NC_CACHE = []


def _run(inputs: dict, trace: bool = False, **kw):
    if not _NC_CACHE:
        _NC_CACHE.append(_build_nc())
    nc = _NC_CACHE[0]
    in_maps = _prep_inputs(**inputs)
    res = run_bass_kernel_spmd(
        nc, in_maps, core_ids=list(range(NCORES)), trace=trace, **kw
    )
    out = np.concatenate(
        [r["out"].astype(np.float32) for r in res.results], axis=0
    )
    return out, res


def kernel(h, v, wh, wv, wg):
    out, _ = _run({"h": h, "v": v, "wh": wh, "wv": wv, "wg": wg})
    return out
